# revision 41
# baseline (speedup 1.0000x reference)
"""Bezier Gaussian-splat raster kernel for 8 Trainium2 NeuronCores.

Problem: control_points [16,4,4,2] f32, sigma scalar f32 ->
raster [16,4,1,512,512] f32 where
  raster[b,s,0,p,q] = sum_t exp(-((y_t-g_p)^2+(x_t-g_q)^2)/(2 sigma^2))
with (x_t,y_t) the cubic Bezier curve sampled at 128 points and
g = arange(512)/512.

Strategy (input-specialized bounding-box culling, data-parallel):
  - The Gaussian has sigma ~10px; each stroke only touches a bbox
    around its curve (+4 sigma margin). Compute/drain/DMA only that
    bbox: rows rounded up to 128-row blocks ("chunks", nch<=4), a
    single x-range of width wx per stroke.
  - SPMD constraint: all 8 cores run ONE program, so bbox shapes are
    static "slots". The 64 strokes are partitioned into 8 slots of 8
    (one per core) minimizing sum_slots max(nch)*max(wx) via local
    search; slot shape = elementwise max. ~15% padding vs exact.
  - Per-stroke pixel offsets (qlo,rlo) are folded into the DATA by
    pre-shifting control points by (qlo,rlo)/RES on the host
    (Bernstein basis sums to 1), so all device APs are static.
  - u = (g - x_px)*sinv/RES built on DVE in f16 (2x mode), exp via
    one batched Derivative_Erf activation per group of strokes
    (Derivative_Erf(u) = 2/sqrt(pi) exp(-u^2)).
  - chunk c of raster = Ay[:,c].T @ Ax on PE (f16 in, f32 PSUM out),
    per-chunk PSUM tiles (1 bank each, 8 in flight).
  - PSUM->SBUF drains (x pi/4, cast f16) are split across ACT, DVE
    and Pool(gpsimd) engines by a greedy makespan balancer.
  - One fp16 output DMA per stroke ([128, nch*wx] packed); host
    reassembles into the zero canvas and upcasts to f32.
"""

import math

import numpy as np

import concourse.bass as bass
import concourse.mybir as mybir
import concourse.tile as tile
from concourse import bacc
from concourse.bass_utils import run_bass_kernel_spmd

RES = 512
STEPS = 128
NK = 4            # control points per stroke
B_FULL = 16
S_FULL = 4
N_CORES = 8
SPC = 8           # strokes (slots) per core
PCH = 128         # rows per chunk

F16 = mybir.dt.float16
F32 = mybir.dt.float32
AF = mybir.ActivationFunctionType
ALU = mybir.AluOpType

PI_OVER_4 = math.pi / 4.0
SQRT2 = math.sqrt(2.0)
AUGW = 2 * SPC + 3 + STEPS

KNOBS = dict(
    margin=4.0,             # bbox margin in sigmas
    exp_mode="auto",        # 'auto' per-slot | 'ubuild' all | 'act' all
    exp_group=4,            # slots per batched exp activation group
    lookahead=2,            # groups of u-build/act emitted ahead of bodies
    dma_defer=True,         # emit each stroke's DMA one body later
    dma_cycle=("sync",),    # engines cycled per-stroke for output DMA issue
    drain_ns=None,          # override _DRAIN_NS for the balancer
    ubuild_ns=None,         # override _UBUILD_NS for the balancer
    upool_bufs=3,
    apool_bufs=4,
    opool_bufs=7,
    pspool_bufs=8,
)

# engine cost model (ns/col, fixed ns) for the greedy balancer.
# HW constraint: Pool (GPSIMD) cannot access PSUM, so drains (PSUM reads)
# are ACT/DVE only; Pool takes u-build work (SBUF->SBUF) instead.
_DRAIN_NS = {
    "act": (0.9, 250.0),
    "dve": (1.042, 125.0),
}
_UBUILD_NS = {
    "dve": (0.52, 62.0),
    "pool": (1.39, 131.0),
}


def _bernstein() -> np.ndarray:
    t = np.linspace(0.0, 1.0, STEPS, dtype=np.float64)
    rows = [math.comb(NK - 1, k) * t ** (NK - 1 - k) * (1.0 - t) ** k
            for k in range(NK)]
    return np.stack(rows).astype(np.float32)  # [4, 128] = feat[k, t]


def geometry(control_points, sigma, margin=None):
    """Host-side bbox + slot partition. Returns a dict:
      slots: tuple of 8 (nch, wx) static shapes
      place: [N_CORES][SPC] of (b, st, qlo, rlo) source stroke + placement
    """
    margin = KNOBS["margin"] if margin is None else margin
    cp = np.asarray(control_points, dtype=np.float32)
    sig = float(np.asarray(sigma).reshape(()))
    feat = _bernstein()
    xy = np.einsum('bski,kt->bsit', cp, feat) * RES  # [16,4,2,128] px
    m = margin * sig * RES

    strokes = []  # (nch, wx, b, st, qlo, rlo)
    for b in range(B_FULL):
        for st in range(S_FULL):
            xs, ys = xy[b, st, 0], xy[b, st, 1]
            qlo = int(max(0.0, np.floor(xs.min() - m)))
            qhi = int(min(RES, np.ceil(xs.max() + m)))
            rlo = int(max(0.0, np.floor(ys.min() - m)))
            rhi = int(min(RES, np.ceil(ys.max() + m)))
            wx = max(2, qhi - qlo)
            wx += wx % 2
            nch = max(1, -(-max(1, rhi - rlo) // PCH))
            strokes.append([nch, wx, b, st, qlo, rlo])

    # partition 64 strokes into 8 slots of 8 minimizing slot-max cost
    def slotcost(grp):
        n = max(strokes[i][0] for i in grp)
        w = max(strokes[i][1] for i in grp)
        return 1.6 * n * w + 1.35 * (w + PCH * n)

    best = None
    for key in (lambda i: (strokes[i][0], strokes[i][1]),
                lambda i: strokes[i][0] * strokes[i][1],
                lambda i: (strokes[i][1], strokes[i][0])):
        order = sorted(range(64), key=key)
        a = [order[8 * j:8 * j + 8] for j in range(8)]
        c = sum(slotcost(g) for g in a)
        if best is None or c < best[0]:
            best = (c, a)
    cur, a = best
    a = [list(g) for g in a]
    rng = np.random.RandomState(12345)
    costs = [slotcost(g) for g in a]
    for _ in range(60000):
        s1 = rng.randint(8)
        s2 = rng.randint(8)
        if s1 == s2:
            continue
        i, j = rng.randint(8), rng.randint(8)
        a[s1][i], a[s2][j] = a[s2][j], a[s1][i]
        c1, c2 = slotcost(a[s1]), slotcost(a[s2])
        d = c1 + c2 - costs[s1] - costs[s2]
        if d <= 0:
            costs[s1], costs[s2] = c1, c2
        else:
            a[s1][i], a[s2][j] = a[s2][j], a[s1][i]

    slots = []
    place = [[None] * SPC for _ in range(N_CORES)]
    for sl in range(8):
        nch = max(strokes[i][0] for i in a[sl])
        wx = max(strokes[i][1] for i in a[sl])
        slots.append((nch, wx))
        for c in range(N_CORES):
            nchi, wxi, b, st, qlo, rlo = strokes[a[sl][c]]
            # shift placement so the slot-sized box stays in-frame
            qlo = max(0, min(qlo, RES - wx))
            rlo = max(0, min(rlo, RES - PCH * nch))
            place[c][sl] = (b, st, qlo, rlo)
    return {"slots": tuple(slots), "place": place}


def _work_plan(slots, exp_mode, G, drain_ns=None, ubuild_ns=None):
    """Choose per-slot exp mode ('ubuild': DVE/Pool builds u, batched exp
    act | 'act': exp act with bias AP, no u-build) and jointly assign drain
    units (PSUM reads: ACT or DVE only) plus u-build segments (SBUF only:
    DVE or Pool) to minimize the makespan. ACT is preloaded by the exp
    activations of the chosen modes. Returns (modes, dplan, uplan)."""
    dns = dict(_DRAIN_NS, **(drain_ns or {}))
    uns = dict(_UBUILD_NS, **(ubuild_ns or {}))
    segw = [wx + PCH * nch for nch, wx in slots]

    if exp_mode == "ubuild":
        masks = [0xFF]
    elif exp_mode == "act":
        masks = [0]
    else:
        masks = range(256)
    best = None
    for mask in masks:
        modes = ["ubuild" if (mask >> s) & 1 else "act"
                 for s in range(SPC)]
        act_pre = 0.833 * sum(segw)
        for g in range(SPC // G):
            if any(modes[s] == "ubuild" for s in range(g * G, (g + 1) * G)):
                act_pre += 185.0
        act_pre += sum(370.0 for s in range(SPC) if modes[s] == "act")
        load = {"act": act_pre, "dve": 0.0, "pool": 0.0}
        units = []  # (width, kind, key); one mixed LPT pass
        for s, (nch, wx) in enumerate(slots):
            for k in range(nch):
                units.append((wx, "drain", (s, k)))
            if modes[s] == "ubuild":
                units.append((wx, "ubuild", (s, "x")))
                units.append((PCH * nch, "ubuild", (s, "y")))
        dplan, uplan = {}, {}
        for w, kind, key in sorted(units, reverse=True):
            tab = dns if kind == "drain" else uns
            e = min(tab, key=lambda e: load[e] + w * tab[e][0] + tab[e][1])
            load[e] += w * tab[e][0] + tab[e][1]
            (dplan if kind == "drain" else uplan)[key] = e
        span = max(load.values())
        if best is None or span < best[0]:
            best = (span, modes, dplan, uplan)
    return best[1], best[2], best[3]


def build_bass(geom, repeats: int = 1, probe: str = "", **over) -> bass.Bass:
    """Build the per-core Bass program for the given geometry. `repeats`
    re-runs the whole stroke loop N times (same outputs) — used by the
    timing harness to estimate steady-state per-iteration HW time."""
    kn = dict(KNOBS, **over)
    slots = list(geom["slots"])
    G = kn["exp_group"]
    assert SPC % max(G, 1) == 0
    modes, dplan, uplan = _work_plan(slots, kn["exp_mode"], G,
                                     kn.get("drain_ns"), kn.get("ubuild_ns"))
    need_gscaled = any(m == "act" for m in modes)
    segw = [wx + PCH * nch for nch, wx in slots]       # act cols per slot
    out_w = max(nch * wx for nch, wx in slots)

    nc = bacc.Bacc("TRN2", target_bir_lowering=False, debug=False,
                   num_devices=N_CORES)

    # One augmented input [4, 147] per core:
    #   [:, 0:16]   shifted control-point coords (x slots 0-7, y slots 0-7)
    #   [0, 16:19]  [sigma, -1/sqrt2, 1/(RES*sqrt2)]
    #   [:, 19:147] Bernstein basis feat[k, t]
    cp_in = nc.dram_tensor("cp_aug", [NK, AUGW], F32, kind="ExternalInput")
    # fp16 output, [slot, psum-partition j, chunk-major packed cols]
    out = nc.dram_tensor("out", [SPC, PCH, out_w], F16,
                         kind="ExternalOutput")

    with tile.TileContext(nc) as tc:
        with tc.tile_pool(name="const", bufs=1) as cpool:
            # Warm the ACT table set (~1.3us load) immediately, overlapping
            # the setup chain: a dep-free Derivative_Erf on a memset tile.
            warm = cpool.tile([1, 1], F32)
            nc.gpsimd.memset(warm[:], 0.0)
            nc.scalar.activation(warm[:], warm[:], AF.Derivative_Erf,
                                 bias=0.0, scale=0.0)

            cp_t = cpool.tile([NK, AUGW], F32)
            nc.sync.dma_start(cp_t[:], cp_in[:])
            feat_t = cp_t[0:NK, 2 * SPC + 3:AUGW]
            g16 = cpool.tile([128, RES], F16)
            nc.gpsimd.iota(g16[:], [[1, RES]], base=0, channel_multiplier=0,
                           allow_small_or_imprecise_dtypes=True)
            if need_gscaled:
                g_tile = cpool.tile([128, RES], F32)
                nc.gpsimd.iota(g_tile[:], [[1, RES]], base=0,
                               channel_multiplier=0,
                               allow_small_or_imprecise_dtypes=True)
            ones_t = cpool.tile([1, 128], F32)
            nc.vector.memset(ones_t[:], 1.0)

            # sinv = 1/(sigma*sqrt2); pm = [-sinv, sinv/RES]
            s1 = cpool.tile([1, 1], F32)
            nc.vector.reciprocal(s1[:], cp_t[0:1, 16:17])
            pm = cpool.tile([1, 2], F32)
            nc.vector.tensor_scalar(pm[:], cp_t[0:1, 17:19], s1[:, 0:1], None,
                                    ALU.mult)

            sinv_sb = cpool.tile([128, 2], F32)   # col0=-sinv col1=sinv/RES
            bias_sb = cpool.tile([128, 2 * SPC], F32)  # -sinv * xy'_s(t)
            xpx = cpool.tile([128, 2 * SPC], F32)      # RES * xy'_s(t)
            with tc.tile_pool(name="spsum", bufs=1, space="PSUM") as spool:
                pbc = spool.tile([128, 2], F32)
                nc.tensor.matmul(pbc[:], lhsT=ones_t[:], rhs=pm[:])
                bps = spool.tile([128, 2 * SPC], F32)
                nc.tensor.matmul(bps[:], lhsT=feat_t, rhs=cp_t[:, 0:2 * SPC])
                nc.vector.tensor_scalar(bias_sb[:], bps[:], pbc[:, 0:1], None,
                                        ALU.mult)
                nc.vector.tensor_scalar(xpx[:], bps[:], float(RES), None,
                                        ALU.mult)
                nc.vector.tensor_copy(sinv_sb[:], pbc[:])
            if need_gscaled:
                gscaled = cpool.tile([128, RES], F32)
                nc.vector.tensor_scalar(gscaled[:], g_tile[:],
                                        sinv_sb[:, 1:2], None, ALU.mult)

            with tc.tile_pool(name="upool", bufs=kn["upool_bufs"]) as upool, \
                 tc.tile_pool(name="apool", bufs=kn["apool_bufs"]) as apool, \
                 tc.tile_pool(name="opool", bufs=kn["opool_bufs"]) as opool, \
                 tc.tile_pool(name="mmpool", bufs=kn["pspool_bufs"],
                              space="PSUM") as mmpool:
                # Software pipeline: emit u-build+act for group g while
                # emitting the compute body (mm/drain/dma) for group g-LA,
                # so next-iteration u-builds never queue behind drains that
                # wait on matmuls.
                from collections import deque

                LA = kn.get("lookahead", 2)
                ngroups = SPC // G
                total = ngroups * repeats
                pending = deque()
                dma_pending = []
                for gi in range(total + LA + 1):
                    if gi < total:
                        g = gi % ngroups
                        gs = list(range(g * G, (g + 1) * G))
                        ub = [s for s in gs if modes[s] == "ubuild"]
                        entries = []
                        if ub:
                            gw = sum(segw[s] for s in ub)
                            ut = upool.tile([128, gw], F16, tag="ut")
                            off = 0
                            parts = {}
                            for s in ub:
                                nch, wx = slots[s]
                                if "noub" not in probe:
                                    ex = nc.vector if uplan[(s, "x")] == \
                                        "dve" else nc.gpsimd
                                    ey = nc.vector if uplan[(s, "y")] == \
                                        "dve" else nc.gpsimd
                                    ex.tensor_scalar(
                                        ut[:, off:off + wx], g16[:, 0:wx],
                                        xpx[:, s:s + 1], sinv_sb[:, 1:2],
                                        ALU.subtract, ALU.mult)
                                    ey.tensor_scalar(
                                        ut[:, off + wx:off + segw[s]],
                                        g16[:, 0:PCH * nch],
                                        xpx[:, SPC + s:SPC + s + 1],
                                        sinv_sb[:, 1:2],
                                        ALU.subtract, ALU.mult)
                                parts[s] = off
                                off += segw[s]
                            axy_u = apool.tile([128, gw], F16, tag="axy")
                            if "noact" not in probe:
                                nc.scalar.activation(axy_u[:], ut[:],
                                                     AF.Derivative_Erf,
                                                     bias=0.0, scale=1.0)
                        for s in gs:
                            if modes[s] == "ubuild":
                                entries.append((s, axy_u, parts[s]))
                                continue
                            nch, wx = slots[s]
                            axy_a = apool.tile([128, segw[s]], F16,
                                               tag="axya")
                            if "noact" not in probe:
                                nc.scalar.activation(
                                    axy_a[:, 0:wx], gscaled[:, 0:wx],
                                    AF.Derivative_Erf,
                                    bias=bias_sb[:, s:s + 1], scale=1.0)
                                nc.scalar.activation(
                                    axy_a[:, wx:segw[s]],
                                    gscaled[:, 0:PCH * nch],
                                    AF.Derivative_Erf,
                                    bias=bias_sb[:, SPC + s:SPC + s + 1],
                                    scale=1.0)
                            entries.append((s, axy_a, 0))
                        pending.append(entries)
                    if len(pending) > LA or (gi >= total and pending):
                        entries = pending.popleft()
                        for op in dma_pending:
                            op()
                        dma_pending = []
                        for s, axy, off in entries:
                            dmas = _emit_stroke(
                                nc, kn, probe, slots, dplan, s,
                                axy, off, mmpool, opool, out)
                            if kn.get("dma_defer", True):
                                dma_pending += dmas
                            else:
                                for op in dmas:
                                    op()
                for op in dma_pending:
                    op()

    nc.finalize()
    return nc


def _emit_stroke(nc, kn, probe, slots, dplan, s, axy, off, mmpool, opool,
                 out):
    """Matmuls + drains for slot s, emitted inline. Returns the DMA thunks
    so the caller can defer them (their drain sems are then satisfied when
    they reach the SP queue head, keeping SP.SEQ occupancy minimal)."""
    nch, wx = slots[s]
    ax = axy[:, off:off + wx]
    ay0 = off + wx
    ps_tiles = []
    for k in range(nch):
        ps = mmpool.tile([128, 512], F32, tag="ps")
        ps_tiles.append(ps)
        if "nomm" not in probe:
            nc.tensor.matmul(ps[:, 0:wx],
                             lhsT=axy[:, ay0 + k * PCH:ay0 + (k + 1) * PCH],
                             rhs=ax)
    ot = opool.tile([128, nch * wx], F16, tag="ot")
    if "nocopy" not in probe:
        # per-chunk drains, ACT or DVE (Pool/GPSIMD cannot read PSUM on HW)
        for k in range(nch):
            dst = ot[:, k * wx:(k + 1) * wx]
            src = ps_tiles[k][:, 0:wx]
            if dplan[(s, k)] == "act":
                nc.scalar.mul(dst, src, PI_OVER_4)
            else:
                nc.vector.tensor_scalar_mul(dst, src, PI_OVER_4)
    if "nodma" in probe:
        return []
    cyc = kn["dma_cycle"]
    eng = getattr(nc, cyc[s % len(cyc)])

    def mkdma(s=s, ot=ot, w=nch * wx, eng=eng):
        eng.dma_start(out[s][:, 0:w], ot[:])
    return [mkdma]


_CACHE: dict = {}


def _get_nc(geom) -> bass.Bass:
    key = (geom["slots"], tuple(sorted(KNOBS.items(),
                                       key=lambda kv: kv[0])))
    key = str(key)
    if key not in _CACHE:
        _CACHE[key] = build_bass(geom)
    return _CACHE[key]


def _in_maps(control_points, sigma, geom) -> list:
    cp = np.asarray(control_points, dtype=np.float32)
    sig = np.float32(np.asarray(sigma).reshape(()))
    isq2 = np.float32(1.0 / SQRT2)
    feat = _bernstein()
    maps = []
    for c in range(N_CORES):
        cp_aug = np.zeros((NK, AUGW), dtype=np.float32)
        for s in range(SPC):
            b, st, qlo, rlo = geom["place"][c][s]
            cp_aug[:, s] = cp[b, st, :, 0] - np.float32(qlo) / RES
            cp_aug[:, SPC + s] = cp[b, st, :, 1] - np.float32(rlo) / RES
        cp_aug[0, 2 * SPC] = sig
        cp_aug[0, 2 * SPC + 1] = -isq2
        cp_aug[0, 2 * SPC + 2] = isq2 / np.float32(RES)
        cp_aug[:, 2 * SPC + 3:] = feat
        maps.append({"cp_aug": np.ascontiguousarray(cp_aug)})
    return maps


_GEOM_CACHE: dict = {}


def geometry_cached(control_points, sigma, margin=None):
    cp = np.asarray(control_points, dtype=np.float32)
    key = (cp.tobytes(), float(np.asarray(sigma).reshape(())), margin)
    if key not in _GEOM_CACHE:
        _GEOM_CACHE[key] = geometry(cp, sigma, margin)
    return _GEOM_CACHE[key]


def run(control_points, sigma, **spmd_kwargs):
    """Run on HW; returns (full_output, BassKernelResults)."""
    geom = geometry_cached(control_points, sigma)
    nc = _get_nc(geom)
    res = run_bass_kernel_spmd(nc, _in_maps(control_points, sigma, geom),
                               core_ids=list(range(N_CORES)), **spmd_kwargs)
    full = np.zeros((B_FULL, S_FULL, RES, RES), dtype=np.float32)
    slots = geom["slots"]
    for c, r in enumerate(res.results):
        o = r["out"]  # [SPC, 128, out_w] f16
        for s in range(SPC):
            nch, wx = slots[s]
            b, st, qlo, rlo = geom["place"][c][s]
            blk = o[s][:, 0:nch * wx].astype(np.float32)
            blk = blk.reshape(PCH, nch, wx).transpose(1, 0, 2)
            full[b, st, rlo:rlo + nch * PCH, qlo:qlo + wx] = \
                blk.reshape(nch * PCH, wx)
    return np.ascontiguousarray(full[:, :, None]), res


def kernel(control_points, sigma):
    return run(control_points, sigma)[0]


# revision 52
# speedup vs baseline: 5.4013x; 5.4013x over previous
"""Bezier Gaussian-splat raster kernel for 8 Trainium2 NeuronCores.

Problem: control_points [16,4,4,2] f32, sigma scalar f32 ->
raster [16,4,1,512,512] f32 where
  raster[b,s,0,p,q] = sum_t exp(-((y_t-g_p)^2+(x_t-g_q)^2)/(2 sigma^2))
with (x_t,y_t) the cubic Bezier curve sampled at 128 points and
g = arange(512)/512.

Strategy (input-specialized bounding-box culling, data-parallel):
  - The Gaussian has sigma ~10px; each stroke only touches a bbox
    around its curve (+4 sigma margin). Compute/drain/DMA only that
    bbox: rows rounded up to 128-row blocks ("chunks", nch<=4), a
    single x-range of width wx per stroke.
  - SPMD constraint: all 8 cores run ONE program, so bbox shapes are
    static "slots". The 64 strokes are partitioned into 8 slots of 8
    (one per core) minimizing sum_slots max(nch)*max(wx) via local
    search; slot shape = elementwise max. ~15% padding vs exact.
  - Per-stroke pixel offsets (qlo,rlo) are folded into the DATA by
    pre-shifting control points by (qlo,rlo)/RES on the host
    (Bernstein basis sums to 1), so all device APs are static.
  - u = (g - x_px)*sinv/RES built on DVE in f16 (2x mode), exp via
    one batched Derivative_Erf activation per group of strokes
    (Derivative_Erf(u) = 2/sqrt(pi) exp(-u^2)).
  - chunk c of raster = Ay[:,c].T @ Ax on PE (f16 in, f32 PSUM out),
    per-chunk PSUM tiles (1 bank each, 8 in flight).
  - PSUM->SBUF drains (x pi/4, cast f16) are split across ACT, DVE
    and Pool(gpsimd) engines by a greedy makespan balancer.
  - One fp16 output DMA per stroke ([128, nch*wx] packed); host
    reassembles into the zero canvas and upcasts to f32.
"""

import math

import numpy as np

import concourse.bass as bass
import concourse.mybir as mybir
import concourse.tile as tile
from concourse import bacc
from concourse.bass_utils import run_bass_kernel_spmd

RES = 512
STEPS = 128
NK = 4            # control points per stroke
B_FULL = 16
S_FULL = 4
N_CORES = 8
SPC = 8           # strokes (slots) per core
PCH = 128         # rows per chunk

F16 = mybir.dt.float16
F32 = mybir.dt.float32
AF = mybir.ActivationFunctionType
ALU = mybir.AluOpType

PI_OVER_4 = math.pi / 4.0
SQRT2 = math.sqrt(2.0)
AUGW = 2 * SPC + 3 + STEPS

KNOBS = dict(
    margin=3.5,             # bbox margin in sigmas
    exp_mode="auto",        # 'auto' per-slot | 'ubuild' all | 'act' all
    exp_group=4,            # slots per batched exp activation group
    lookahead=2,            # groups of u-build/act emitted ahead of bodies
    dma_defer=True,         # emit each stroke's DMA one body later
    dma_cycle=("sync",),    # engines cycled per-stroke for output DMA issue
    drain_ns=None,          # override _DRAIN_NS for the balancer
    ubuild_ns=None,         # override _UBUILD_NS for the balancer
    upool_bufs=3,
    apool_bufs=4,
    opool_bufs=7,
    pspool_bufs=8,
)

# engine cost model (ns/col, fixed ns) for the greedy balancer.
# HW constraint: Pool (GPSIMD) cannot access PSUM, so drains (PSUM reads)
# are ACT/DVE only; Pool takes u-build work (SBUF->SBUF) instead.
_DRAIN_NS = {
    "act": (0.9, 250.0),
    "dve": (1.042, 125.0),
}
# Pool (GPSIMD) software tensor ops measured ~6-8x slower on real HW than
# the cost model's 0.6-efficiency guess (57us vs 10us full-kernel) — keep
# Pool out of the hot loop entirely.
_UBUILD_NS = {
    "dve": (0.52, 62.0),
}


def _bernstein() -> np.ndarray:
    t = np.linspace(0.0, 1.0, STEPS, dtype=np.float64)
    rows = [math.comb(NK - 1, k) * t ** (NK - 1 - k) * (1.0 - t) ** k
            for k in range(NK)]
    return np.stack(rows).astype(np.float32)  # [4, 128] = feat[k, t]


def geometry(control_points, sigma, margin=None):
    """Host-side bbox + slot partition. Returns a dict:
      slots: tuple of 8 (nch, wx) static shapes
      place: [N_CORES][SPC] of (b, st, qlo, rlo) source stroke + placement
    """
    margin = KNOBS["margin"] if margin is None else margin
    cp = np.asarray(control_points, dtype=np.float32)
    sig = float(np.asarray(sigma).reshape(()))
    feat = _bernstein()
    xy = np.einsum('bski,kt->bsit', cp, feat) * RES  # [16,4,2,128] px
    m = margin * sig * RES

    strokes = []  # (nch, wx, b, st, qlo, rlo)
    for b in range(B_FULL):
        for st in range(S_FULL):
            xs, ys = xy[b, st, 0], xy[b, st, 1]
            qlo = int(max(0.0, np.floor(xs.min() - m)))
            qhi = int(min(RES, np.ceil(xs.max() + m)))
            rlo = int(max(0.0, np.floor(ys.min() - m)))
            rhi = int(min(RES, np.ceil(ys.max() + m)))
            wx = max(2, qhi - qlo)
            wx += wx % 2
            nch = max(1, -(-max(1, rhi - rlo) // PCH))
            strokes.append([nch, wx, b, st, qlo, rlo])

    # partition 64 strokes into 8 slots of 8 minimizing slot-max cost
    def slotcost(grp):
        n = max(strokes[i][0] for i in grp)
        w = max(strokes[i][1] for i in grp)
        return 1.6 * n * w + 1.35 * (w + PCH * n)

    best = None
    for key in (lambda i: (strokes[i][0], strokes[i][1]),
                lambda i: strokes[i][0] * strokes[i][1],
                lambda i: (strokes[i][1], strokes[i][0])):
        order = sorted(range(64), key=key)
        a = [order[8 * j:8 * j + 8] for j in range(8)]
        c = sum(slotcost(g) for g in a)
        if best is None or c < best[0]:
            best = (c, a)
    cur, a = best
    a = [list(g) for g in a]
    rng = np.random.RandomState(12345)
    costs = [slotcost(g) for g in a]
    for _ in range(60000):
        s1 = rng.randint(8)
        s2 = rng.randint(8)
        if s1 == s2:
            continue
        i, j = rng.randint(8), rng.randint(8)
        a[s1][i], a[s2][j] = a[s2][j], a[s1][i]
        c1, c2 = slotcost(a[s1]), slotcost(a[s2])
        d = c1 + c2 - costs[s1] - costs[s2]
        if d <= 0:
            costs[s1], costs[s2] = c1, c2
        else:
            a[s1][i], a[s2][j] = a[s2][j], a[s1][i]

    slots = []
    place = [[None] * SPC for _ in range(N_CORES)]
    for sl in range(8):
        nch = max(strokes[i][0] for i in a[sl])
        wx = max(strokes[i][1] for i in a[sl])
        slots.append((nch, wx))
        for c in range(N_CORES):
            nchi, wxi, b, st, qlo, rlo = strokes[a[sl][c]]
            # shift placement so the slot-sized box stays in-frame
            qlo = max(0, min(qlo, RES - wx))
            rlo = max(0, min(rlo, RES - PCH * nch))
            place[c][sl] = (b, st, qlo, rlo)
    return {"slots": tuple(slots), "place": place}


def _work_plan(slots, exp_mode, G, drain_ns=None, ubuild_ns=None):
    """Choose per-slot exp mode ('ubuild': DVE/Pool builds u, batched exp
    act | 'act': exp act with bias AP, no u-build) and jointly assign drain
    units (PSUM reads: ACT or DVE only) plus u-build segments (SBUF only:
    DVE or Pool) to minimize the makespan. ACT is preloaded by the exp
    activations of the chosen modes. Returns (modes, dplan, uplan)."""
    dns = dict(_DRAIN_NS, **(drain_ns or {}))
    uns = dict(_UBUILD_NS, **(ubuild_ns or {}))
    segw = [wx + PCH * nch for nch, wx in slots]

    if exp_mode == "ubuild":
        masks = [0xFF]
    elif exp_mode == "act":
        masks = [0]
    elif exp_mode == "pemm":
        masks = ["pemm"]
    else:
        masks = range(256)
    best = None
    for mask in masks:
        if mask == "pemm":
            modes = ["pemm"] * SPC
            act_pre = sum(
                2 * max(wx, PCH * nch) * 0.833 + 185.0
                for nch, wx in slots)
        else:
            modes = ["ubuild" if (mask >> s) & 1 else "act"
                     for s in range(SPC)]
            act_pre = 0.833 * sum(segw)
            for g in range(SPC // G):
                if any(modes[s] == "ubuild"
                       for s in range(g * G, (g + 1) * G)):
                    act_pre += 185.0
            act_pre += sum(370.0 for s in range(SPC) if modes[s] == "act")
        load = {"act": act_pre, "dve": 0.0, "pool": 0.0}
        units = []  # (width, kind, key); one mixed LPT pass
        for s, (nch, wx) in enumerate(slots):
            for k in range(nch):
                units.append((wx, "drain", (s, k)))
            if modes[s] == "ubuild":
                units.append((wx, "ubuild", (s, "x")))
                units.append((PCH * nch, "ubuild", (s, "y")))
        dplan, uplan = {}, {}
        for w, kind, key in sorted(units, reverse=True):
            tab = dns if kind == "drain" else uns
            e = min(tab, key=lambda e: load[e] + w * tab[e][0] + tab[e][1])
            load[e] += w * tab[e][0] + tab[e][1]
            (dplan if kind == "drain" else uplan)[key] = e
        span = max(load.values())
        if best is None or span < best[0]:
            best = (span, modes, dplan, uplan)
    return best[1], best[2], best[3]


def build_bass(geom, repeats: int = 1, probe: str = "", **over) -> bass.Bass:
    """Build the per-core Bass program for the given geometry. `repeats`
    re-runs the whole stroke loop N times (same outputs) — used by the
    timing harness to estimate steady-state per-iteration HW time."""
    kn = dict(KNOBS, **over)
    slots = list(geom["slots"])
    G = kn["exp_group"]
    assert SPC % max(G, 1) == 0
    modes, dplan, uplan = _work_plan(slots, kn["exp_mode"], G,
                                     kn.get("drain_ns"), kn.get("ubuild_ns"))
    need_gscaled = any(m == "act" for m in modes)
    need_pemm = any(m == "pemm" for m in modes)
    if need_pemm:
        G = 1  # pemm pipelines per-stroke (u lives in PSUM)
    segw = [wx + PCH * nch for nch, wx in slots]       # act cols per slot
    out_w = max(nch * wx for nch, wx in slots)

    nc = bacc.Bacc("TRN2", target_bir_lowering=False, debug=False,
                   num_devices=N_CORES)

    # One augmented input [4, 147] per core:
    #   [:, 0:16]   shifted control-point coords (x slots 0-7, y slots 0-7)
    #   [0, 16:19]  [sigma, -1/sqrt2, 1/(RES*sqrt2)]
    #   [:, 19:147] Bernstein basis feat[k, t]
    cp_in = nc.dram_tensor("cp_aug", [NK, AUGW], F32, kind="ExternalInput")
    # fp16 output, [slot, psum-partition j, chunk-major packed cols]
    out = nc.dram_tensor("out", [SPC, PCH, out_w], F16,
                         kind="ExternalOutput")

    with tile.TileContext(nc) as tc:
        with tc.tile_pool(name="const", bufs=1) as cpool:
            # Warm the ACT table set (~1.3us load) immediately, overlapping
            # the setup chain: a dep-free Derivative_Erf on a memset tile.
            warm = cpool.tile([1, 1], F32)
            nc.gpsimd.memset(warm[:], 0.0)
            nc.scalar.activation(warm[:], warm[:], AF.Derivative_Erf,
                                 bias=0.0, scale=0.0)

            cp_t = cpool.tile([NK, AUGW], F32)
            nc.sync.dma_start(cp_t[:], cp_in[:])
            feat_t = cp_t[0:NK, 2 * SPC + 3:AUGW]
            g16 = cpool.tile([128, RES], F16)
            nc.gpsimd.iota(g16[:], [[1, RES]], base=0, channel_multiplier=0,
                           allow_small_or_imprecise_dtypes=True)
            if need_gscaled:
                g_tile = cpool.tile([128, RES], F32)
                nc.gpsimd.iota(g_tile[:], [[1, RES]], base=0,
                               channel_multiplier=0,
                               allow_small_or_imprecise_dtypes=True)
            ones_t = cpool.tile([1, 128], F32)
            nc.vector.memset(ones_t[:], 1.0)

            # sinv = 1/(sigma*sqrt2); pm = [-sinv, sinv/RES]
            s1 = cpool.tile([1, 1], F32)
            nc.vector.reciprocal(s1[:], cp_t[0:1, 16:17])
            pm = cpool.tile([1, 2], F32)
            nc.vector.tensor_scalar(pm[:], cp_t[0:1, 17:19], s1[:, 0:1], None,
                                    ALU.mult)

            sinv_sb = cpool.tile([128, 2], F32)   # col0=-sinv col1=sinv/RES
            bias_sb = cpool.tile([128, 2 * SPC], F32)  # -sinv * xy'_s(t)
            xpx = cpool.tile([128, 2 * SPC], F32)      # RES * xy'_s(t)
            with tc.tile_pool(name="spsum", bufs=1, space="PSUM") as spool:
                pbc = spool.tile([128, 2], F32)
                nc.tensor.matmul(pbc[:], lhsT=ones_t[:], rhs=pm[:])
                bps = spool.tile([128, 2 * SPC], F32)
                nc.tensor.matmul(bps[:], lhsT=feat_t, rhs=cp_t[:, 0:2 * SPC])
                nc.vector.tensor_scalar(bias_sb[:], bps[:], pbc[:, 0:1], None,
                                        ALU.mult)
                nc.vector.tensor_scalar(xpx[:], bps[:], float(RES), None,
                                        ALU.mult)
                nc.vector.tensor_copy(sinv_sb[:], pbc[:])
            if need_gscaled:
                gscaled = cpool.tile([128, RES], F32)
                nc.vector.tensor_scalar(gscaled[:], g_tile[:],
                                        sinv_sb[:, 1:2], None, ALU.mult)
            if need_pemm:
                BF16 = mybir.dt.bfloat16
                # rhs [4, 512]: rows = [(s*g)_hi, (s*g)_lo, 1, 1] bf16,
                # an exact hi/lo bf16 split of s*g (error ~2^-17 rel)
                gr4 = cpool.tile([4, RES], BF16)
                g1 = cpool.tile([1, RES], F32)
                nc.gpsimd.iota(g1[:], [[1, RES]], base=0,
                               channel_multiplier=0,
                               allow_small_or_imprecise_dtypes=True)
                gs1 = cpool.tile([1, RES], F32)
                nc.vector.tensor_scalar(gs1[:], g1[:], sinv_sb[0:1, 1:2],
                                        None, ALU.mult)
                ghi = cpool.tile([1, RES], BF16)
                nc.vector.tensor_copy(ghi[:], gs1[:])
                glo = cpool.tile([1, RES], BF16)
                nc.vector.tensor_tensor(glo[:], gs1[:], ghi[:],
                                        ALU.subtract)
                ones2 = cpool.tile([2, RES], BF16)
                nc.vector.memset(ones2[:], 1.0)
                # cross-partition placement via DMA (engine ops cannot
                # start off partition 0 / shift partitions)
                nc.sync.dma_start(gr4[0:1, :], ghi[:])
                nc.sync.dma_start(gr4[1:2, :], glo[:])
                nc.sync.dma_start(gr4[2:4, :], ones2[:])
                # lhsT [4, 16*128]: per seg i: rows = [1, 1, b_hi_i, b_lo_i]
                # where b_i(t) = -sinv * xy'_i(t)
                ob = cpool.tile([4, 2 * SPC * PCH], BF16)
                nc.vector.memset(ob[0:2, :], 1.0)
                with tc.tile_pool(name="ppsum", bufs=1,
                                  space="PSUM") as ppool:
                    bT = ppool.tile([2 * SPC, PCH], F32)
                    nc.tensor.matmul(bT[:], lhsT=cp_t[:, 0:2 * SPC],
                                     rhs=feat_t)
                    bN = cpool.tile([2 * SPC, PCH], F32)
                    nc.vector.tensor_scalar(bN[:], bT[:],
                                            sinv_sb[0:2 * SPC, 0:1],
                                            None, ALU.mult)
                bhi = cpool.tile([2 * SPC, PCH], BF16)
                nc.vector.tensor_copy(bhi[:], bN[:])
                blo = cpool.tile([2 * SPC, PCH], BF16)
                nc.vector.tensor_tensor(blo[:], bN[:], bhi[:], ALU.subtract)
                nc.sync.dma_start(ob[2:3, :], bhi[:])
                nc.sync.dma_start(ob[3:4, :], blo[:])

            with tc.tile_pool(name="upool", bufs=kn["upool_bufs"]) as upool, \
                 tc.tile_pool(name="apool", bufs=kn["apool_bufs"]) as apool, \
                 tc.tile_pool(name="opool", bufs=kn["opool_bufs"]) as opool, \
                 tc.tile_pool(name="mmpool", bufs=kn["pspool_bufs"],
                              space="PSUM") as mmpool:
                # Software pipeline: emit u-build+act for group g while
                # emitting the compute body (mm/drain/dma) for group g-LA,
                # so next-iteration u-builds never queue behind drains that
                # wait on matmuls.
                from collections import deque

                LA = kn.get("lookahead", 2)
                ngroups = SPC // G
                total = ngroups * repeats
                pending = deque()
                dma_pending = []
                for gi in range(total + LA + 1):
                    if gi < total:
                        g = gi % ngroups
                        gs = list(range(g * G, (g + 1) * G))
                        ub = [s for s in gs if modes[s] == "ubuild"]
                        entries = []
                        if ub:
                            gw = sum(segw[s] for s in ub)
                            ut = upool.tile([128, gw], F16, tag="ut")
                            off = 0
                            parts = {}
                            for s in ub:
                                nch, wx = slots[s]
                                if "noub" not in probe:
                                    ex = nc.vector if uplan[(s, "x")] == \
                                        "dve" else nc.gpsimd
                                    ey = nc.vector if uplan[(s, "y")] == \
                                        "dve" else nc.gpsimd
                                    ex.tensor_scalar(
                                        ut[:, off:off + wx], g16[:, 0:wx],
                                        xpx[:, s:s + 1], sinv_sb[:, 1:2],
                                        ALU.subtract, ALU.mult)
                                    ey.tensor_scalar(
                                        ut[:, off + wx:off + segw[s]],
                                        g16[:, 0:PCH * nch],
                                        xpx[:, SPC + s:SPC + s + 1],
                                        sinv_sb[:, 1:2],
                                        ALU.subtract, ALU.mult)
                                parts[s] = off
                                off += segw[s]
                            axy_u = apool.tile([128, gw], F16, tag="axy")
                            if "noact" not in probe:
                                nc.scalar.activation(axy_u[:], ut[:],
                                                     AF.Derivative_Erf,
                                                     bias=0.0, scale=1.0)
                        for s in gs:
                            nch, wx = slots[s]
                            if modes[s] == "ubuild":
                                entries.append((s, axy_u, parts[s],
                                                parts[s] + wx))
                                continue
                            if modes[s] == "pemm":
                                W = max(wx, PCH * nch)
                                ups = mmpool.tile([128, 1024], F32,
                                                  tag="ups", bufs=2)
                                if "noub" not in probe:
                                    nc.tensor.matmul(
                                        ups[:, 0:wx],
                                        lhsT=ob[:, s * PCH:(s + 1) * PCH],
                                        rhs=gr4[:, 0:wx])
                                    nc.tensor.matmul(
                                        ups[:, 512:512 + PCH * nch],
                                        lhsT=ob[:, (SPC + s) * PCH:
                                                (SPC + s + 1) * PCH],
                                        rhs=gr4[:, 0:PCH * nch])
                                axy_p = apool.tile([128, 2 * W], F16,
                                                   tag="axyp")
                                if "noact" not in probe:
                                    u3 = ups[:].rearrange(
                                        "p (c w) -> p c w", c=2)[:, :, 0:W]
                                    a3 = axy_p[:].rearrange(
                                        "p (c w) -> p c w", c=2)
                                    nc.scalar.activation(
                                        a3, u3, AF.Derivative_Erf,
                                        bias=0.0, scale=1.0)
                                entries.append((s, axy_p, 0, W))
                                continue
                            axy_a = apool.tile([128, segw[s]], F16,
                                               tag="axya")
                            if "noact" not in probe:
                                nc.scalar.activation(
                                    axy_a[:, 0:wx], gscaled[:, 0:wx],
                                    AF.Derivative_Erf,
                                    bias=bias_sb[:, s:s + 1], scale=1.0)
                                nc.scalar.activation(
                                    axy_a[:, wx:segw[s]],
                                    gscaled[:, 0:PCH * nch],
                                    AF.Derivative_Erf,
                                    bias=bias_sb[:, SPC + s:SPC + s + 1],
                                    scale=1.0)
                            entries.append((s, axy_a, 0, wx))
                        pending.append(entries)
                    if len(pending) > LA or (gi >= total and pending):
                        entries = pending.popleft()
                        for op in dma_pending:
                            op()
                        dma_pending = []
                        for s, axy, xoff, ayoff in entries:
                            dmas = _emit_stroke(
                                nc, kn, probe, slots, dplan, s,
                                axy, xoff, ayoff, mmpool, opool, out,
                                ps_bufs=4 if need_pemm else None)
                            if kn.get("dma_defer", True):
                                dma_pending += dmas
                            else:
                                for op in dmas:
                                    op()
                for op in dma_pending:
                    op()

    nc.finalize()
    return nc


def _emit_stroke(nc, kn, probe, slots, dplan, s, axy, xoff, ay0, mmpool,
                 opool, out, ps_bufs=None):
    """Matmuls + drains for slot s, emitted inline. Returns the DMA thunks
    so the caller can defer them (their drain sems are then satisfied when
    they reach the SP queue head, keeping SP.SEQ occupancy minimal)."""
    nch, wx = slots[s]
    ax = axy[:, xoff:xoff + wx]
    ps_tiles = []
    for k in range(nch):
        ps = mmpool.tile([128, 512], F32, tag="ps", bufs=ps_bufs)
        ps_tiles.append(ps)
        if "nomm" not in probe:
            nc.tensor.matmul(ps[:, 0:wx],
                             lhsT=axy[:, ay0 + k * PCH:ay0 + (k + 1) * PCH],
                             rhs=ax)
    ot = opool.tile([128, nch * wx], F16, tag="ot")
    if "nocopy" not in probe:
        # per-chunk drains, ACT or DVE (Pool/GPSIMD cannot read PSUM on HW)
        for k in range(nch):
            dst = ot[:, k * wx:(k + 1) * wx]
            src = ps_tiles[k][:, 0:wx]
            if dplan[(s, k)] == "act":
                nc.scalar.mul(dst, src, PI_OVER_4)
            else:
                nc.vector.tensor_scalar_mul(dst, src, PI_OVER_4)
    if "nodma" in probe:
        return []
    cyc = kn["dma_cycle"]
    eng = getattr(nc, cyc[s % len(cyc)])

    def mkdma(s=s, ot=ot, w=nch * wx, eng=eng):
        eng.dma_start(out[s][:, 0:w], ot[:])
    return [mkdma]


_CACHE: dict = {}


def _get_nc(geom) -> bass.Bass:
    key = (geom["slots"], tuple(sorted(KNOBS.items(),
                                       key=lambda kv: kv[0])))
    key = str(key)
    if key not in _CACHE:
        _CACHE[key] = build_bass(geom)
    return _CACHE[key]


def _in_maps(control_points, sigma, geom) -> list:
    cp = np.asarray(control_points, dtype=np.float32)
    sig = np.float32(np.asarray(sigma).reshape(()))
    isq2 = np.float32(1.0 / SQRT2)
    feat = _bernstein()
    maps = []
    for c in range(N_CORES):
        cp_aug = np.zeros((NK, AUGW), dtype=np.float32)
        for s in range(SPC):
            b, st, qlo, rlo = geom["place"][c][s]
            cp_aug[:, s] = cp[b, st, :, 0] - np.float32(qlo) / RES
            cp_aug[:, SPC + s] = cp[b, st, :, 1] - np.float32(rlo) / RES
        cp_aug[0, 2 * SPC] = sig
        cp_aug[0, 2 * SPC + 1] = -isq2
        cp_aug[0, 2 * SPC + 2] = isq2 / np.float32(RES)
        cp_aug[:, 2 * SPC + 3:] = feat
        maps.append({"cp_aug": np.ascontiguousarray(cp_aug)})
    return maps


_GEOM_CACHE: dict = {}


def geometry_cached(control_points, sigma, margin=None):
    cp = np.asarray(control_points, dtype=np.float32)
    key = (cp.tobytes(), float(np.asarray(sigma).reshape(())), margin)
    if key not in _GEOM_CACHE:
        _GEOM_CACHE[key] = geometry(cp, sigma, margin)
    return _GEOM_CACHE[key]


def run(control_points, sigma, **spmd_kwargs):
    """Run on HW; returns (full_output, BassKernelResults)."""
    geom = geometry_cached(control_points, sigma)
    nc = _get_nc(geom)
    res = run_bass_kernel_spmd(nc, _in_maps(control_points, sigma, geom),
                               core_ids=list(range(N_CORES)), **spmd_kwargs)
    full = np.zeros((B_FULL, S_FULL, RES, RES), dtype=np.float32)
    slots = geom["slots"]
    for c, r in enumerate(res.results):
        o = r["out"]  # [SPC, 128, out_w] f16
        for s in range(SPC):
            nch, wx = slots[s]
            b, st, qlo, rlo = geom["place"][c][s]
            blk = o[s][:, 0:nch * wx].astype(np.float32)
            blk = blk.reshape(PCH, nch, wx).transpose(1, 0, 2)
            full[b, st, rlo:rlo + nch * PCH, qlo:qlo + wx] = \
                blk.reshape(nch * PCH, wx)
    return np.ascontiguousarray(full[:, :, None]), res


def kernel(control_points, sigma):
    return run(control_points, sigma)[0]


# revision 54
# speedup vs baseline: 8.9390x; 1.6550x over previous
"""Bezier Gaussian-splat raster kernel for 8 Trainium2 NeuronCores.

Problem: control_points [16,4,4,2] f32, sigma scalar f32 ->
raster [16,4,1,512,512] f32 where
  raster[b,s,0,p,q] = sum_t exp(-((y_t-g_p)^2+(x_t-g_q)^2)/(2 sigma^2))
with (x_t,y_t) the cubic Bezier curve sampled at 128 points and
g = arange(512)/512.

Strategy (input-specialized bounding-box culling, data-parallel):
  - The Gaussian has sigma ~10px; each stroke only touches a bbox
    around its curve (+margin sigmas). Compute/drain/DMA only that
    bbox: rows rounded up to 128-row blocks ("chunks", nch<=4), a
    single x-range of width wx per stroke.
  - SPMD constraint: all 8 cores run ONE program, so bbox shapes are
    static "slots". The 64 strokes are partitioned into 8 slots of 8
    (one per core) minimizing sum_slots max(nch)*max(wx) via local
    search; slot shape = elementwise max. ~15% padding vs exact.
  - Per-stroke pixel offsets (qlo,rlo) are folded into the DATA by
    pre-shifting control points by (qlo,rlo)/RES on the host
    (Bernstein basis sums to 1), so all device APs are static.
  - exp via Derivative_Erf(u) = 2/sqrt(pi) exp(-u^2). Per-slot mode
    (planner-chosen): 'act' = bias/scale APs fold u into the act;
    'ubuild' = DVE builds u in f16 (2x mode), batched group act;
    'pemm' = K=4 bf16 hi/lo-split matmul builds u in PSUM.
  - chunk c of raster = Ay[:,c].T @ Ax on PE (f16 in, f32 PSUM out),
    per-chunk PSUM tiles (1 bank each, 8 in flight).
  - PSUM->SBUF drains (x pi/4, cast f16) split across ACT and DVE by
    a greedy makespan balancer (GPSIMD cannot access PSUM on HW, and
    its software tensor ops measured ~7x slower than the cost model
    claims, so Pool is excluded from the hot loop entirely).
  - One fp16 output DMA per stroke ([128, nch*wx] packed); host
    reassembles into the zero canvas and upcasts to f32.
"""

import math

import numpy as np

import concourse.bass as bass
import concourse.mybir as mybir
import concourse.tile as tile
from concourse import bacc
from concourse.bass_utils import run_bass_kernel_spmd

RES = 512
STEPS = 128
NK = 4            # control points per stroke
B_FULL = 16
S_FULL = 4
N_CORES = 8
SPC = 8           # strokes (slots) per core
PCH = 128         # rows per chunk

F16 = mybir.dt.float16
F32 = mybir.dt.float32
AF = mybir.ActivationFunctionType
ALU = mybir.AluOpType

PI_OVER_4 = math.pi / 4.0
SQRT2 = math.sqrt(2.0)
AUGW = 2 * SPC + 3 + STEPS

KNOBS = dict(
    margin=3.0,             # bbox margin in sigmas (rel_l2 ~6e-4 at 3.0)
    exp_mode="auto",        # 'auto' per-slot | 'ubuild' | 'act' | 'pemm'
    exp_group=4,            # slots per batched exp activation group
    lookahead=2,            # groups of u-build/act emitted ahead of bodies
    dma_defer=True,         # emit each stroke's DMA one body later
    dma_cycle=("sync",),    # engines cycled per-stroke for output DMA issue
    # HW-tuned: bias drains heavily toward DVE (ACT keeps only the
    # largest units) — measured fastest on hardware A/B
    drain_ns={"act": (0.9, 2000.0)},
    ubuild_ns=None,         # override _UBUILD_NS for the balancer
    upool_bufs=3,
    apool_bufs=4,
    opool_bufs=7,
    pspool_bufs=8,
)

# engine cost model (ns/col, fixed ns) for the greedy balancer.
# HW constraint: Pool (GPSIMD) cannot access PSUM, so drains (PSUM reads)
# are ACT/DVE only; Pool takes u-build work (SBUF->SBUF) instead.
_DRAIN_NS = {
    "act": (0.9, 250.0),
    "dve": (1.042, 125.0),
}
# Pool (GPSIMD) software tensor ops measured ~6-8x slower on real HW than
# the cost model's 0.6-efficiency guess (57us vs 10us full-kernel) — keep
# Pool out of the hot loop entirely.
_UBUILD_NS = {
    "dve": (0.52, 62.0),
}


def _bernstein() -> np.ndarray:
    t = np.linspace(0.0, 1.0, STEPS, dtype=np.float64)
    rows = [math.comb(NK - 1, k) * t ** (NK - 1 - k) * (1.0 - t) ** k
            for k in range(NK)]
    return np.stack(rows).astype(np.float32)  # [4, 128] = feat[k, t]


def geometry(control_points, sigma, margin=None):
    """Host-side bbox + slot partition. Returns a dict:
      slots: tuple of 8 (nch, wx) static shapes
      place: [N_CORES][SPC] of (b, st, qlo, rlo) source stroke + placement
    """
    margin = KNOBS["margin"] if margin is None else margin
    cp = np.asarray(control_points, dtype=np.float32)
    sig = float(np.asarray(sigma).reshape(()))
    feat = _bernstein()
    xy = np.einsum('bski,kt->bsit', cp, feat) * RES  # [16,4,2,128] px
    m = margin * sig * RES

    strokes = []  # (nch, wx, b, st, qlo, rlo)
    for b in range(B_FULL):
        for st in range(S_FULL):
            xs, ys = xy[b, st, 0], xy[b, st, 1]
            qlo = int(max(0.0, np.floor(xs.min() - m)))
            qhi = int(min(RES, np.ceil(xs.max() + m)))
            rlo = int(max(0.0, np.floor(ys.min() - m)))
            rhi = int(min(RES, np.ceil(ys.max() + m)))
            wx = max(2, qhi - qlo)
            wx += wx % 2
            nch = max(1, -(-max(1, rhi - rlo) // PCH))
            strokes.append([nch, wx, b, st, qlo, rlo])

    # partition 64 strokes into 8 slots of 8 minimizing slot-max cost
    def slotcost(grp):
        n = max(strokes[i][0] for i in grp)
        w = max(strokes[i][1] for i in grp)
        return 1.6 * n * w + 1.35 * (w + PCH * n)

    best = None
    for key in (lambda i: (strokes[i][0], strokes[i][1]),
                lambda i: strokes[i][0] * strokes[i][1],
                lambda i: (strokes[i][1], strokes[i][0])):
        order = sorted(range(64), key=key)
        a = [order[8 * j:8 * j + 8] for j in range(8)]
        c = sum(slotcost(g) for g in a)
        if best is None or c < best[0]:
            best = (c, a)
    cur, a = best
    a = [list(g) for g in a]
    rng = np.random.RandomState(12345)
    costs = [slotcost(g) for g in a]
    for _ in range(60000):
        s1 = rng.randint(8)
        s2 = rng.randint(8)
        if s1 == s2:
            continue
        i, j = rng.randint(8), rng.randint(8)
        a[s1][i], a[s2][j] = a[s2][j], a[s1][i]
        c1, c2 = slotcost(a[s1]), slotcost(a[s2])
        d = c1 + c2 - costs[s1] - costs[s2]
        if d <= 0:
            costs[s1], costs[s2] = c1, c2
        else:
            a[s1][i], a[s2][j] = a[s2][j], a[s1][i]

    slots = []
    place = [[None] * SPC for _ in range(N_CORES)]
    for sl in range(8):
        nch = max(strokes[i][0] for i in a[sl])
        wx = max(strokes[i][1] for i in a[sl])
        slots.append((nch, wx))
        for c in range(N_CORES):
            nchi, wxi, b, st, qlo, rlo = strokes[a[sl][c]]
            # shift placement so the slot-sized box stays in-frame
            qlo = max(0, min(qlo, RES - wx))
            rlo = max(0, min(rlo, RES - PCH * nch))
            place[c][sl] = (b, st, qlo, rlo)
    return {"slots": tuple(slots), "place": place}


def _work_plan(slots, exp_mode, G, drain_ns=None, ubuild_ns=None):
    """Choose per-slot exp mode ('ubuild': DVE/Pool builds u, batched exp
    act | 'act': exp act with bias AP, no u-build) and jointly assign drain
    units (PSUM reads: ACT or DVE only) plus u-build segments (SBUF only:
    DVE or Pool) to minimize the makespan. ACT is preloaded by the exp
    activations of the chosen modes. Returns (modes, dplan, uplan)."""
    dns = dict(_DRAIN_NS, **(drain_ns or {}))
    uns = dict(_UBUILD_NS, **(ubuild_ns or {}))
    segw = [wx + PCH * nch for nch, wx in slots]

    if exp_mode == "ubuild":
        masks = [0xFF]
    elif exp_mode == "act":
        masks = [0]
    elif exp_mode == "pemm":
        masks = ["pemm"]
    else:
        masks = range(256)
    best = None
    for mask in masks:
        if mask == "pemm":
            modes = ["pemm"] * SPC
            act_pre = sum(
                2 * max(wx, PCH * nch) * 0.833 + 185.0
                for nch, wx in slots)
        else:
            modes = ["ubuild" if (mask >> s) & 1 else "act"
                     for s in range(SPC)]
            act_pre = 0.833 * sum(segw)
            for g in range(SPC // G):
                if any(modes[s] == "ubuild"
                       for s in range(g * G, (g + 1) * G)):
                    act_pre += 185.0
            act_pre += sum(370.0 for s in range(SPC) if modes[s] == "act")
        load = {"act": act_pre, "dve": 0.0, "pool": 0.0}
        units = []  # (width, kind, key); one mixed LPT pass
        for s, (nch, wx) in enumerate(slots):
            for k in range(nch):
                units.append((wx, "drain", (s, k)))
            if modes[s] == "ubuild":
                units.append((wx, "ubuild", (s, "x")))
                units.append((PCH * nch, "ubuild", (s, "y")))
        dplan, uplan = {}, {}
        for w, kind, key in sorted(units, reverse=True):
            tab = dns if kind == "drain" else uns
            e = min(tab, key=lambda e: load[e] + w * tab[e][0] + tab[e][1])
            load[e] += w * tab[e][0] + tab[e][1]
            (dplan if kind == "drain" else uplan)[key] = e
        span = max(load.values())
        if best is None or span < best[0]:
            best = (span, modes, dplan, uplan)
    return best[1], best[2], best[3]


def build_bass(geom, repeats: int = 1, probe: str = "", **over) -> bass.Bass:
    """Build the per-core Bass program for the given geometry. `repeats`
    re-runs the whole stroke loop N times (same outputs) — used by the
    timing harness to estimate steady-state per-iteration HW time."""
    kn = dict(KNOBS, **over)
    slots = list(geom["slots"])
    G = kn["exp_group"]
    assert SPC % max(G, 1) == 0
    modes, dplan, uplan = _work_plan(slots, kn["exp_mode"], G,
                                     kn.get("drain_ns"), kn.get("ubuild_ns"))
    need_gscaled = any(m == "act" for m in modes)
    need_pemm = any(m == "pemm" for m in modes)
    if need_pemm:
        G = 1  # pemm pipelines per-stroke (u lives in PSUM)
    segw = [wx + PCH * nch for nch, wx in slots]       # act cols per slot
    out_w = max(nch * wx for nch, wx in slots)

    nc = bacc.Bacc("TRN2", target_bir_lowering=False, debug=False,
                   num_devices=N_CORES)

    # One augmented input [4, 147] per core:
    #   [:, 0:16]   shifted control-point coords (x slots 0-7, y slots 0-7)
    #   [0, 16:19]  [sigma, -1/sqrt2, 1/(RES*sqrt2)]
    #   [:, 19:147] Bernstein basis feat[k, t]
    cp_in = nc.dram_tensor("cp_aug", [NK, AUGW], F32, kind="ExternalInput")
    # fp16 output, [slot, psum-partition j, chunk-major packed cols]
    out = nc.dram_tensor("out", [SPC, PCH, out_w], F16,
                         kind="ExternalOutput")

    with tile.TileContext(nc) as tc:
        with tc.tile_pool(name="const", bufs=1) as cpool:
            # Warm the ACT table set (~1.3us load) immediately, overlapping
            # the setup chain: a dep-free Derivative_Erf on a memset tile.
            warm = cpool.tile([1, 1], F32)
            nc.gpsimd.memset(warm[:], 0.0)
            nc.scalar.activation(warm[:], warm[:], AF.Derivative_Erf,
                                 bias=0.0, scale=0.0)

            cp_t = cpool.tile([NK, AUGW], F32)
            nc.sync.dma_start(cp_t[:], cp_in[:])
            feat_t = cp_t[0:NK, 2 * SPC + 3:AUGW]
            g16 = cpool.tile([128, RES], F16)
            nc.gpsimd.iota(g16[:], [[1, RES]], base=0, channel_multiplier=0,
                           allow_small_or_imprecise_dtypes=True)
            if need_gscaled:
                g_tile = cpool.tile([128, RES], F32)
                nc.gpsimd.iota(g_tile[:], [[1, RES]], base=0,
                               channel_multiplier=0,
                               allow_small_or_imprecise_dtypes=True)
            ones_t = cpool.tile([1, 128], F32)
            nc.vector.memset(ones_t[:], 1.0)

            # sinv = 1/(sigma*sqrt2); pm = [-sinv, sinv/RES]
            s1 = cpool.tile([1, 1], F32)
            nc.vector.reciprocal(s1[:], cp_t[0:1, 16:17])
            pm = cpool.tile([1, 2], F32)
            nc.vector.tensor_scalar(pm[:], cp_t[0:1, 17:19], s1[:, 0:1], None,
                                    ALU.mult)

            sinv_sb = cpool.tile([128, 2], F32)   # col0=-sinv col1=sinv/RES
            bias_sb = cpool.tile([128, 2 * SPC], F32)  # -sinv * xy'_s(t)
            xpx = cpool.tile([128, 2 * SPC], F32)      # RES * xy'_s(t)
            with tc.tile_pool(name="spsum", bufs=1, space="PSUM") as spool:
                pbc = spool.tile([128, 2], F32)
                nc.tensor.matmul(pbc[:], lhsT=ones_t[:], rhs=pm[:])
                bps = spool.tile([128, 2 * SPC], F32)
                nc.tensor.matmul(bps[:], lhsT=feat_t, rhs=cp_t[:, 0:2 * SPC])
                nc.vector.tensor_scalar(bias_sb[:], bps[:], pbc[:, 0:1], None,
                                        ALU.mult)
                nc.vector.tensor_scalar(xpx[:], bps[:], float(RES), None,
                                        ALU.mult)
                nc.vector.tensor_copy(sinv_sb[:], pbc[:])
            if need_gscaled:
                gscaled = cpool.tile([128, RES], F32)
                nc.vector.tensor_scalar(gscaled[:], g_tile[:],
                                        sinv_sb[:, 1:2], None, ALU.mult)
            if need_pemm:
                BF16 = mybir.dt.bfloat16
                # rhs [4, 512]: rows = [(s*g)_hi, (s*g)_lo, 1, 1] bf16,
                # an exact hi/lo bf16 split of s*g (error ~2^-17 rel)
                gr4 = cpool.tile([4, RES], BF16)
                g1 = cpool.tile([1, RES], F32)
                nc.gpsimd.iota(g1[:], [[1, RES]], base=0,
                               channel_multiplier=0,
                               allow_small_or_imprecise_dtypes=True)
                gs1 = cpool.tile([1, RES], F32)
                nc.vector.tensor_scalar(gs1[:], g1[:], sinv_sb[0:1, 1:2],
                                        None, ALU.mult)
                ghi = cpool.tile([1, RES], BF16)
                nc.vector.tensor_copy(ghi[:], gs1[:])
                glo = cpool.tile([1, RES], BF16)
                nc.vector.tensor_tensor(glo[:], gs1[:], ghi[:],
                                        ALU.subtract)
                ones2 = cpool.tile([2, RES], BF16)
                nc.vector.memset(ones2[:], 1.0)
                # cross-partition placement via DMA (engine ops cannot
                # start off partition 0 / shift partitions)
                nc.sync.dma_start(gr4[0:1, :], ghi[:])
                nc.sync.dma_start(gr4[1:2, :], glo[:])
                nc.sync.dma_start(gr4[2:4, :], ones2[:])
                # lhsT [4, 16*128]: per seg i: rows = [1, 1, b_hi_i, b_lo_i]
                # where b_i(t) = -sinv * xy'_i(t)
                ob = cpool.tile([4, 2 * SPC * PCH], BF16)
                nc.vector.memset(ob[0:2, :], 1.0)
                with tc.tile_pool(name="ppsum", bufs=1,
                                  space="PSUM") as ppool:
                    bT = ppool.tile([2 * SPC, PCH], F32)
                    nc.tensor.matmul(bT[:], lhsT=cp_t[:, 0:2 * SPC],
                                     rhs=feat_t)
                    bN = cpool.tile([2 * SPC, PCH], F32)
                    nc.vector.tensor_scalar(bN[:], bT[:],
                                            sinv_sb[0:2 * SPC, 0:1],
                                            None, ALU.mult)
                bhi = cpool.tile([2 * SPC, PCH], BF16)
                nc.vector.tensor_copy(bhi[:], bN[:])
                blo = cpool.tile([2 * SPC, PCH], BF16)
                nc.vector.tensor_tensor(blo[:], bN[:], bhi[:], ALU.subtract)
                nc.sync.dma_start(ob[2:3, :], bhi[:])
                nc.sync.dma_start(ob[3:4, :], blo[:])

            with tc.tile_pool(name="upool", bufs=kn["upool_bufs"]) as upool, \
                 tc.tile_pool(name="apool", bufs=kn["apool_bufs"]) as apool, \
                 tc.tile_pool(name="opool", bufs=kn["opool_bufs"]) as opool, \
                 tc.tile_pool(name="mmpool", bufs=kn["pspool_bufs"],
                              space="PSUM") as mmpool:
                # Software pipeline: emit u-build+act for group g while
                # emitting the compute body (mm/drain/dma) for group g-LA,
                # so next-iteration u-builds never queue behind drains that
                # wait on matmuls.
                from collections import deque

                LA = kn.get("lookahead", 2)
                ngroups = SPC // G
                total = ngroups * repeats
                pending = deque()
                dma_pending = []
                for gi in range(total + LA + 1):
                    if gi < total:
                        g = gi % ngroups
                        gs = list(range(g * G, (g + 1) * G))
                        ub = [s for s in gs if modes[s] == "ubuild"]
                        entries = []
                        if ub:
                            gw = sum(segw[s] for s in ub)
                            ut = upool.tile([128, gw], F16, tag="ut")
                            off = 0
                            parts = {}
                            for s in ub:
                                nch, wx = slots[s]
                                if "noub" not in probe:
                                    ex = nc.vector if uplan[(s, "x")] == \
                                        "dve" else nc.gpsimd
                                    ey = nc.vector if uplan[(s, "y")] == \
                                        "dve" else nc.gpsimd
                                    ex.tensor_scalar(
                                        ut[:, off:off + wx], g16[:, 0:wx],
                                        xpx[:, s:s + 1], sinv_sb[:, 1:2],
                                        ALU.subtract, ALU.mult)
                                    ey.tensor_scalar(
                                        ut[:, off + wx:off + segw[s]],
                                        g16[:, 0:PCH * nch],
                                        xpx[:, SPC + s:SPC + s + 1],
                                        sinv_sb[:, 1:2],
                                        ALU.subtract, ALU.mult)
                                parts[s] = off
                                off += segw[s]
                            axy_u = apool.tile([128, gw], F16, tag="axy")
                            if "noact" not in probe:
                                nc.scalar.activation(axy_u[:], ut[:],
                                                     AF.Derivative_Erf,
                                                     bias=0.0, scale=1.0)
                        for s in gs:
                            nch, wx = slots[s]
                            if modes[s] == "ubuild":
                                entries.append((s, axy_u, parts[s],
                                                parts[s] + wx))
                                continue
                            if modes[s] == "pemm":
                                W = max(wx, PCH * nch)
                                ups = mmpool.tile([128, 1024], F32,
                                                  tag="ups", bufs=2)
                                if "noub" not in probe:
                                    nc.tensor.matmul(
                                        ups[:, 0:wx],
                                        lhsT=ob[:, s * PCH:(s + 1) * PCH],
                                        rhs=gr4[:, 0:wx])
                                    nc.tensor.matmul(
                                        ups[:, 512:512 + PCH * nch],
                                        lhsT=ob[:, (SPC + s) * PCH:
                                                (SPC + s + 1) * PCH],
                                        rhs=gr4[:, 0:PCH * nch])
                                axy_p = apool.tile([128, 2 * W], F16,
                                                   tag="axyp")
                                if "noact" not in probe:
                                    u3 = ups[:].rearrange(
                                        "p (c w) -> p c w", c=2)[:, :, 0:W]
                                    a3 = axy_p[:].rearrange(
                                        "p (c w) -> p c w", c=2)
                                    nc.scalar.activation(
                                        a3, u3, AF.Derivative_Erf,
                                        bias=0.0, scale=1.0)
                                entries.append((s, axy_p, 0, W))
                                continue
                            axy_a = apool.tile([128, segw[s]], F16,
                                               tag="axya")
                            if "noact" not in probe:
                                nc.scalar.activation(
                                    axy_a[:, 0:wx], gscaled[:, 0:wx],
                                    AF.Derivative_Erf,
                                    bias=bias_sb[:, s:s + 1], scale=1.0)
                                nc.scalar.activation(
                                    axy_a[:, wx:segw[s]],
                                    gscaled[:, 0:PCH * nch],
                                    AF.Derivative_Erf,
                                    bias=bias_sb[:, SPC + s:SPC + s + 1],
                                    scale=1.0)
                            entries.append((s, axy_a, 0, wx))
                        pending.append(entries)
                    if len(pending) > LA or (gi >= total and pending):
                        entries = pending.popleft()
                        for op in dma_pending:
                            op()
                        dma_pending = []
                        for s, axy, xoff, ayoff in entries:
                            dmas = _emit_stroke(
                                nc, kn, probe, slots, dplan, s,
                                axy, xoff, ayoff, mmpool, opool, out,
                                ps_bufs=4 if need_pemm else None)
                            if kn.get("dma_defer", True):
                                dma_pending += dmas
                            else:
                                for op in dmas:
                                    op()
                for op in dma_pending:
                    op()

    nc.finalize()
    return nc


def _emit_stroke(nc, kn, probe, slots, dplan, s, axy, xoff, ay0, mmpool,
                 opool, out, ps_bufs=None):
    """Matmuls + drains for slot s, emitted inline. Returns the DMA thunks
    so the caller can defer them (their drain sems are then satisfied when
    they reach the SP queue head, keeping SP.SEQ occupancy minimal)."""
    nch, wx = slots[s]
    ax = axy[:, xoff:xoff + wx]
    ps_tiles = []
    for k in range(nch):
        ps = mmpool.tile([128, 512], F32, tag="ps", bufs=ps_bufs)
        ps_tiles.append(ps)
        if "nomm" not in probe:
            nc.tensor.matmul(ps[:, 0:wx],
                             lhsT=axy[:, ay0 + k * PCH:ay0 + (k + 1) * PCH],
                             rhs=ax)
    ot = opool.tile([128, nch * wx], F16, tag="ot")
    if "nocopy" not in probe:
        # per-chunk drains, ACT or DVE (Pool/GPSIMD cannot read PSUM on HW)
        for k in range(nch):
            dst = ot[:, k * wx:(k + 1) * wx]
            src = ps_tiles[k][:, 0:wx]
            if dplan[(s, k)] == "act":
                nc.scalar.mul(dst, src, PI_OVER_4)
            else:
                nc.vector.tensor_scalar_mul(dst, src, PI_OVER_4)
    if "nodma" in probe:
        return []
    cyc = kn["dma_cycle"]
    eng = getattr(nc, cyc[s % len(cyc)])

    def mkdma(s=s, ot=ot, w=nch * wx, eng=eng):
        eng.dma_start(out[s][:, 0:w], ot[:])
    return [mkdma]


_CACHE: dict = {}


def _get_nc(geom) -> bass.Bass:
    key = (geom["slots"], tuple(sorted(KNOBS.items(),
                                       key=lambda kv: kv[0])))
    key = str(key)
    if key not in _CACHE:
        _CACHE[key] = build_bass(geom)
    return _CACHE[key]


def _in_maps(control_points, sigma, geom) -> list:
    cp = np.asarray(control_points, dtype=np.float32)
    sig = np.float32(np.asarray(sigma).reshape(()))
    isq2 = np.float32(1.0 / SQRT2)
    feat = _bernstein()
    maps = []
    for c in range(N_CORES):
        cp_aug = np.zeros((NK, AUGW), dtype=np.float32)
        for s in range(SPC):
            b, st, qlo, rlo = geom["place"][c][s]
            cp_aug[:, s] = cp[b, st, :, 0] - np.float32(qlo) / RES
            cp_aug[:, SPC + s] = cp[b, st, :, 1] - np.float32(rlo) / RES
        cp_aug[0, 2 * SPC] = sig
        cp_aug[0, 2 * SPC + 1] = -isq2
        cp_aug[0, 2 * SPC + 2] = isq2 / np.float32(RES)
        cp_aug[:, 2 * SPC + 3:] = feat
        maps.append({"cp_aug": np.ascontiguousarray(cp_aug)})
    return maps


_GEOM_CACHE: dict = {}


def geometry_cached(control_points, sigma, margin=None):
    cp = np.asarray(control_points, dtype=np.float32)
    key = (cp.tobytes(), float(np.asarray(sigma).reshape(())), margin)
    if key not in _GEOM_CACHE:
        _GEOM_CACHE[key] = geometry(cp, sigma, margin)
    return _GEOM_CACHE[key]


def run(control_points, sigma, **spmd_kwargs):
    """Run on HW; returns (full_output, BassKernelResults)."""
    geom = geometry_cached(control_points, sigma)
    nc = _get_nc(geom)
    res = run_bass_kernel_spmd(nc, _in_maps(control_points, sigma, geom),
                               core_ids=list(range(N_CORES)), **spmd_kwargs)
    full = np.zeros((B_FULL, S_FULL, RES, RES), dtype=np.float32)
    slots = geom["slots"]
    for c, r in enumerate(res.results):
        o = r["out"]  # [SPC, 128, out_w] f16
        for s in range(SPC):
            nch, wx = slots[s]
            b, st, qlo, rlo = geom["place"][c][s]
            blk = o[s][:, 0:nch * wx].astype(np.float32)
            blk = blk.reshape(PCH, nch, wx).transpose(1, 0, 2)
            full[b, st, rlo:rlo + nch * PCH, qlo:qlo + wx] = \
                blk.reshape(nch * PCH, wx)
    return np.ascontiguousarray(full[:, :, None]), res


def kernel(control_points, sigma):
    return run(control_points, sigma)[0]
